# revision 14
# baseline (speedup 1.0000x reference)
"""Trainium2 Bass kernel for nn_CNN_align (TPS-warp masked correlation).

Strategy
--------
Data-parallel over batch: core b handles sample b (B == n_cores == 8).

Host side (cheap, ~tiny): replicate the reference's TPS grid computation
bit-exactly on the jax CPU backend -> warp grids gx, gy [B,48,48]. From
these, build the binary row/col masks and observe that for each output row
block (b, i, :) the mask cy[b,i,k,l] is nonzero only for k in a narrow
window (<= 13 wide after unioning over the batch). Everything outside that
band of the [B,H,W,H,W] output is zero -- and the run_bass_kernel_spmd /
PJRT path hands the kernel zero-initialized (donated) output buffers, so
the kernel only writes the band (~22% of the output) and reads only the
matching band of corr_scores.

Device side (per core, static python loop over 24 i-pairs):
  corr band  [96, nk*48] f32  <- HWDGE DMA   (i-pair x j partitions)
  mask band  [96, nk*48] f32  <- one up-front SWDGE cast-DMA (u8 in HBM)
  stage = corr * mask         <- DVE tensor_tensor
  colsum[:, t] = reduce(stage)<- DVE tensor_reduce
  out band   <- HWDGE DMA
Final: reduce colsums, ones-matmul across partitions -> per-sample sum.
"""

import numpy as np

H = W = 48
B = 8
NPAIR = H // 2  # 24 i-pairs per sample
PARTS = 96      # (2 i-values) x (48 j-values)
THRESH = 1.0

SRC = np.array([[0.0, 0.0], [0.5, 0.0], [1.0, 0.0],
                [0.0, 0.5], [0.5, 0.5], [1.0, 0.5],
                [0.0, 1.0], [5.0, 1.0], [1.0, 1.0]], dtype=np.float32)

LAST_RESULTS = None  # debugging hook for test.py


def _tps_grids_cpu(geo_parameters):
    """Bit-exact replication of the reference _tps_grid pipeline on jax CPU."""
    import jax
    import jax.numpy as jnp

    def _u(r):
        return r * r * jnp.log(r + 1e-6)

    def _pd(a, b):
        return jnp.sqrt(jnp.sum((a[:, None, :] - b[None, :, :]) ** 2, -1))

    def _tps_fit(c, v):
        n = c.shape[0]
        U = _u(_pd(c, c))
        P = jnp.concatenate([jnp.ones((n, 1), c.dtype), c], 1)
        A = jnp.zeros((n + 3, n + 3), c.dtype)
        A = A.at[:n, :n].set(U).at[:n, n:].set(P).at[n:, :n].set(P.T)
        rhs = jnp.concatenate([v, jnp.zeros((3,), c.dtype)])
        theta = jnp.linalg.solve(A, rhs)
        return theta[1:]

    def _tps_z(x, c, theta):
        w, a = theta[:-3], theta[-3:]
        w = jnp.concatenate([-jnp.sum(w, keepdims=True), w])
        bb = _u(_pd(x, c)) @ w
        return a[0] + a[1] * x[:, 0] + a[2] * x[:, 1] + bb

    def _tps_grid(mv, Hh, Ww):
        src = jnp.asarray(SRC)
        dst = src + mv
        delta = src - dst
        th_x = _tps_fit(dst, delta[:, 0])
        th_y = _tps_fit(dst, delta[:, 1])
        ug = jnp.stack(jnp.meshgrid(jnp.linspace(0.0, 1.0, Ww, dtype=jnp.float32),
                                    jnp.linspace(0.0, 1.0, Hh, dtype=jnp.float32)), -1)
        xf = ug.reshape(-1, 2)
        dx = _tps_z(xf, dst, th_x).reshape(Hh, Ww)
        dy = _tps_z(xf, dst, th_y).reshape(Hh, Ww)
        return jnp.stack([dx, dy], -1) + ug

    cpu = jax.devices("cpu")[0]
    with jax.default_device(cpu):
        grids = jax.vmap(lambda mv: _tps_grid(mv, H, W))(jnp.asarray(geo_parameters))
        gx = np.asarray(grids[..., 0] * (H - 1))
        gy = np.asarray(grids[..., 1] * (W - 1))
    return gx, gy


NQUAD = H // 4          # 12 iterations, each covering 4 i-rows
GROUP_QUADS = [1, 2, 3, 6]  # lazy mask groups: first is tiny so compute starts fast


def _build_plan(gx, gy):
    """Per-i-quad k-windows (unioned over batch) + per-core band masks.

    Each iteration covers 4 consecutive i's as [96 partitions = (a, j),
    free = (c, k, l)] with i = i0 + 2c + a; both i-pairs share one k-window.
    """
    ax = np.arange(W, dtype=np.float32)
    ay = np.arange(H, dtype=np.float32)
    cx = (np.abs(ax[None, :, None, None] - gx[:, None, :, :]) <= THRESH)
    cy = (np.abs(ay[None, :, None, None] - gy[:, None, :, :]) <= THRESH)

    any_l = cy.any(axis=3)  # [B, i, k]
    windows = []
    for t in range(NQUAD):
        sel = any_l[:, 4 * t:4 * t + 4, :].any(axis=(0, 1))  # [k]
        nz = np.flatnonzero(sel)
        if len(nz) == 0:
            windows.append((0, 1))
        else:
            windows.append((int(nz.min()), int(nz.max() - nz.min() + 1)))

    # band masks as u8, partition-major per group of quads; free layout per
    # quad is (c, k, l) matching the device AP
    group_F = []
    chunks = []
    q0 = 0
    for nq in GROUP_QUADS:
        ts = range(q0, q0 + nq)
        Fg = sum(2 * windows[t][1] * W for t in ts)
        group_F.append(Fg)
        block = np.empty((B, PARTS, Fg), dtype=np.uint8)
        off = 0
        for t in ts:
            k0, nk = windows[t]
            # m[b, a, j, c, k, l] = cy[b, 4t+2c+a, k, l] & cx[b, j, k, l]
            cyq = cy[:, 4 * t:4 * t + 4, k0:k0 + nk, :]     # [B, i4, nk, 48]
            cyq = cyq.reshape(B, 2, 2, nk, W)               # [B, c, a, nk, 48]
            cxb = cx[:, :, k0:k0 + nk, :]                   # [B, 48, nk, 48]
            m = (cyq[:, :, :, None, :, :] & cxb[:, None, None, :, :, :])
            # m: [B, c, a, j, nk, 48] -> [B, (a j), (c nk 48)]
            m = m.transpose(0, 2, 3, 1, 4, 5).reshape(B, PARTS, 2 * nk * W)
            block[:, :, off:off + 2 * nk * W] = m.astype(np.uint8)
            off += 2 * nk * W
        chunks.append(block.reshape(B, PARTS * Fg))
        q0 += nq
    mask_flat = np.concatenate(chunks, axis=1)
    return windows, mask_flat, group_F


def _build_program(windows, group_F):
    import concourse.mybir as mybir
    from concourse import bacc, tile

    f32 = mybir.dt.float32
    u8 = mybir.dt.uint8
    sumF = sum(group_F)
    nc = bacc.Bacc(None, target_bir_lowering=False, num_devices=B)
    corr_in = nc.declare_dram_parameter("corr", [H, W, H, W], f32, isOutput=False)
    mask_in = nc.declare_dram_parameter("mask", [PARTS * sumF], u8, isOutput=False)
    out_t = nc.declare_dram_parameter("out", [H, W, H, W], f32, isOutput=True)
    sum_t = nc.declare_dram_parameter("sums", [1, 1], f32, isOutput=True)

    # quad -> (group idx, col offset in group tile)
    q2group = []
    for g, nq in enumerate(GROUP_QUADS):
        off = 0
        for _ in range(nq):
            q2group.append((g, off))
            t = len(q2group) - 1
            off += 2 * windows[t][1] * W

    with tile.TileContext(nc) as tc:
        with tc.tile_pool(name="const", bufs=1) as cpool, \
             tc.tile_pool(name="work", bufs=8) as pool, \
             tc.tile_pool(name="fini", bufs=1) as fpool, \
             tc.tile_pool(name="psum", bufs=1, space="PSUM") as psump:
            mtiles = []
            for g in range(len(GROUP_QUADS)):
                mtile = cpool.tile([PARTS, group_F[g]], u8, tag=f"masks{g}")
                mtiles.append(mtile)
            colsums = cpool.tile([PARTS, NQUAD], f32, tag="colsums")

            goffs = np.cumsum([0] + [PARTS * F for F in group_F])
            emitted_groups = set()

            def emit_group(g):
                if g in emitted_groups:
                    return
                emitted_groups.add(g)
                nc.sync.dma_start(
                    out=mtiles[g][:],
                    in_=mask_in[int(goffs[g]):int(goffs[g + 1])]
                    .rearrange("(p f) -> p f", p=PARTS))

            for t, (k0, nk) in enumerate(windows):
                g, off = q2group[t]
                i0 = 4 * t
                Ft = 2 * nk * W
                band_in = corr_in[i0:i0 + 4, :, k0:k0 + nk, :] \
                    .rearrange("(c a) j k l -> (a j) c (k l)", c=2, a=2)
                corr_tile = pool.tile([PARTS, Ft], f32, tag="corr")
                nc.sync.dma_start(out=corr_tile[:], in_=band_in)
                emit_group(g)          # mask group lands right after this corr
                if t + 1 < NQUAD:
                    emit_group(q2group[t + 1][0])  # prefetch next group early
                stage = pool.tile([PARTS, Ft], f32, tag="stage")
                nc.vector.tensor_tensor(
                    out=stage[:], in0=corr_tile[:],
                    in1=mtiles[g][:, off:off + Ft],
                    op=mybir.AluOpType.mult)
                # per-sample sums: split between ACT (accum regs) and DVE
                if t % 2 == 0:
                    scratch = pool.tile([PARTS, Ft], f32, tag="scratch")
                    nc.scalar.activation(
                        out=scratch[:], in_=stage[:],
                        func=mybir.ActivationFunctionType.Copy,
                        accum_out=colsums[:, t:t + 1])
                else:
                    nc.vector.tensor_reduce(
                        out=colsums[:, t:t + 1], in_=stage[:],
                        axis=mybir.AxisListType.X, op=mybir.AluOpType.add)
                band_out = out_t[i0:i0 + 4, :, k0:k0 + nk, :] \
                    .rearrange("(c a) j k l -> (a j) c (k l)", c=2, a=2)
                nc.scalar.dma_start(out=band_out, in_=stage[:])

            rowacc = fpool.tile([PARTS, 1], f32, tag="rowacc")
            nc.vector.tensor_reduce(out=rowacc[:], in_=colsums[:],
                                    axis=mybir.AxisListType.X,
                                    op=mybir.AluOpType.add)
            ones = cpool.tile([PARTS, 1], f32, tag="ones")
            nc.vector.memset(ones[:], 1.0)
            ps = psump.tile([1, 1], f32, tag="ps")
            nc.tensor.matmul(ps[:], ones[:], rowacc[:], start=True, stop=True)
            fin = fpool.tile([1, 1], f32, tag="fin")
            nc.vector.tensor_copy(out=fin[:], in_=ps[:])
            nc.sync.dma_start(out=sum_t[:], in_=fin[:])

    nc.finalize()
    return nc


def kernel(geo_parameters, corr_scores):
    from concourse.bass_utils import run_bass_kernel_spmd

    geo_parameters = np.asarray(geo_parameters)
    corr_scores = np.ascontiguousarray(np.asarray(corr_scores, dtype=np.float32))

    gx, gy = _tps_grids_cpu(geo_parameters)
    windows, mask_flat, group_F = _build_plan(gx, gy)
    nc = _build_program(windows, group_F)

    in_maps = [{"corr": corr_scores[b], "mask": mask_flat[b]} for b in range(B)]
    res = run_bass_kernel_spmd(nc, in_maps, list(range(B)))
    global LAST_RESULTS
    LAST_RESULTS = res

    inlier = np.stack([res.results[b]["out"] for b in range(B)], axis=0)
    sums = np.array([res.results[b]["sums"][0, 0] for b in range(B)],
                    dtype=np.float32)
    return inlier, sums


# revision 19
# speedup vs baseline: 1.2017x; 1.2017x over previous
"""Trainium2 Bass kernel for nn_CNN_align (TPS-warp masked correlation).

Strategy
--------
Data-parallel over batch: core b handles sample b (B == n_cores == 8).

Host side (cheap): replicate the reference's TPS grid computation bit-exactly
on the jax CPU backend -> warp grids gx, gy [B,48,48]. The combined mask
cy & cx is nonzero only in a narrow k-band per output row block (b, i, :)
(window <= 13 after unioning over the batch). Everything outside the band is
zero -- and the run_bass_kernel_spmd / PJRT path donates zero-initialized
output buffers, so the kernel only writes the band (~22% of the output) and
reads only the matching band of corr_scores. The product masks ride along as
uint8 (DVE converts on read).

Device side (per core, 24 i-pair iterations):
  corr band  [96, nk*48] f32  <- HWDGE DMA on sync   (i-pair x j partitions)
  mask band  [96, nk*48] u8   <- 4 up-front HWDGE DMAs on sync
  stage = corr * mask         <- DVE tensor_tensor (u8 operand converts)
  colsums[:, t]               <- ACT accum (even t) / DVE reduce (odd t)
  out band   <- HWDGE DMA on scalar
Iterations alternate SBUF partition offset 0/32 so concurrent DMAs cover all
16 SBUF ports (a fixed [0:96) range would cap DMA at 12/16 of fabric BW).
Final: reduce colsums, ones-matmul across partitions -> per-sample sum.
"""

import numpy as np

H = W = 48
B = 8
NPAIR = H // 2  # 24 i-pairs per sample
PARTS = 96      # (2 i-values) x (48 j-values)
THRESH = 1.0

SRC = np.array([[0.0, 0.0], [0.5, 0.0], [1.0, 0.0],
                [0.0, 0.5], [0.5, 0.5], [1.0, 0.5],
                [0.0, 1.0], [5.0, 1.0], [1.0, 1.0]], dtype=np.float32)

LAST_RESULTS = None  # debugging hook for test.py


def _tps_grids_cpu(geo_parameters):
    """Bit-exact replication of the reference _tps_grid pipeline on jax CPU."""
    import jax
    import jax.numpy as jnp

    def _u(r):
        return r * r * jnp.log(r + 1e-6)

    def _pd(a, b):
        return jnp.sqrt(jnp.sum((a[:, None, :] - b[None, :, :]) ** 2, -1))

    def _tps_fit(c, v):
        n = c.shape[0]
        U = _u(_pd(c, c))
        P = jnp.concatenate([jnp.ones((n, 1), c.dtype), c], 1)
        A = jnp.zeros((n + 3, n + 3), c.dtype)
        A = A.at[:n, :n].set(U).at[:n, n:].set(P).at[n:, :n].set(P.T)
        rhs = jnp.concatenate([v, jnp.zeros((3,), c.dtype)])
        theta = jnp.linalg.solve(A, rhs)
        return theta[1:]

    def _tps_z(x, c, theta):
        w, a = theta[:-3], theta[-3:]
        w = jnp.concatenate([-jnp.sum(w, keepdims=True), w])
        bb = _u(_pd(x, c)) @ w
        return a[0] + a[1] * x[:, 0] + a[2] * x[:, 1] + bb

    def _tps_grid(mv, Hh, Ww):
        src = jnp.asarray(SRC)
        dst = src + mv
        delta = src - dst
        th_x = _tps_fit(dst, delta[:, 0])
        th_y = _tps_fit(dst, delta[:, 1])
        ug = jnp.stack(jnp.meshgrid(jnp.linspace(0.0, 1.0, Ww, dtype=jnp.float32),
                                    jnp.linspace(0.0, 1.0, Hh, dtype=jnp.float32)), -1)
        xf = ug.reshape(-1, 2)
        dx = _tps_z(xf, dst, th_x).reshape(Hh, Ww)
        dy = _tps_z(xf, dst, th_y).reshape(Hh, Ww)
        return jnp.stack([dx, dy], -1) + ug

    cpu = jax.devices("cpu")[0]
    with jax.default_device(cpu):
        grids = jax.vmap(lambda mv: _tps_grid(mv, H, W))(jnp.asarray(geo_parameters))
        gx = np.asarray(grids[..., 0] * (H - 1))
        gy = np.asarray(grids[..., 1] * (W - 1))
    return gx, gy


NTILE = (H * W) // 128      # 18 tiles of 128 consecutive (i,j) rows
GROUP_TILES = [2, 4, 5, 7]  # mask DMA grouping (first smaller -> compute starts early)


def _build_plan(gx, gy):
    """Per-tile k-windows (unioned over batch + the tile's i-range) + masks.

    The output viewed as [(i j), k, l] is tiled as 18 blocks of 128
    consecutive (i,j) rows; each block reads/writes only its k-window.
    """
    ax = np.arange(W, dtype=np.float32)
    ay = np.arange(H, dtype=np.float32)
    cx = (np.abs(ax[None, :, None, None] - gx[:, None, :, :]) <= THRESH)
    cy = (np.abs(ay[None, :, None, None] - gy[:, None, :, :]) <= THRESH)

    any_l = cy.any(axis=3)  # [B, i, k]
    windows = []
    for s in range(NTILE):
        ilo = (s * 128) // W
        ihi = ((s + 1) * 128 - 1) // W
        sel = any_l[:, ilo:ihi + 1, :].any(axis=(0, 1))
        nz = np.flatnonzero(sel)
        if len(nz) == 0:
            windows.append((0, 1))
        else:
            windows.append((int(nz.min()), int(nz.max() - nz.min() + 1)))

    ii = np.arange(H * W) // W   # row -> i
    jj = np.arange(H * W) % W    # row -> j
    group_F = []
    chunks = []
    s0 = 0
    for ng in GROUP_TILES:
        ss = range(s0, s0 + ng)
        Fg = sum(windows[s][1] * W for s in ss)
        group_F.append(Fg)
        block = np.empty((B, 128, Fg), dtype=np.uint8)
        off = 0
        for s in ss:
            k0, nk = windows[s]
            rows = np.arange(s * 128, (s + 1) * 128)
            # m[b, p, k, l] = cy[b, i(p), k0+k, l] & cx[b, j(p), k0+k, l]
            m = (cy[:, ii[rows], k0:k0 + nk, :] & cx[:, jj[rows], k0:k0 + nk, :])
            block[:, :, off:off + nk * W] = \
                m.reshape(B, 128, nk * W).astype(np.uint8)
            off += nk * W
        chunks.append(block.reshape(B, 128 * Fg))
        s0 += ng
    mask_flat = np.concatenate(chunks, axis=1)
    return windows, mask_flat, group_F


def _build_program(windows, group_F):
    import concourse.mybir as mybir
    from concourse import bacc, tile

    f32 = mybir.dt.float32
    u8 = mybir.dt.uint8
    sumF = sum(group_F)
    nc = bacc.Bacc(None, target_bir_lowering=False, num_devices=B)
    corr_in = nc.declare_dram_parameter("corr", [H, W, H, W], f32, isOutput=False)
    mask_in = nc.declare_dram_parameter("mask", [128 * sumF], u8, isOutput=False)
    out_t = nc.declare_dram_parameter("out", [H, W, H, W], f32, isOutput=True)
    sum_t = nc.declare_dram_parameter("sums", [1, 1], f32, isOutput=True)

    corr_flat = corr_in.rearrange("i j k l -> (i j) k l")
    out_flat = out_t.rearrange("i j k l -> (i j) k l")

    # tile -> (group idx, col offset in group tile)
    s2group = []
    for g, ng in enumerate(GROUP_TILES):
        off = 0
        for _ in range(ng):
            s2group.append((g, off))
            s = len(s2group) - 1
            off += windows[s][1] * W

    with tile.TileContext(nc) as tc:
        with tc.tile_pool(name="const", bufs=1) as cpool, \
             tc.tile_pool(name="work", bufs=6) as pool, \
             tc.tile_pool(name="fini", bufs=1) as fpool, \
             tc.tile_pool(name="psum", bufs=1, space="PSUM") as psump:
            mtiles = []
            for g in range(len(GROUP_TILES)):
                mtile = cpool.tile([128, group_F[g]], u8, tag=f"masks{g}")
                mtiles.append(mtile)
            colsums = cpool.tile([128, NTILE], f32, tag="colsums")

            goffs = np.cumsum([0] + [128 * F for F in group_F])
            emitted = set()

            def emit_group(g):
                if g in emitted:
                    return
                emitted.add(g)
                nc.sync.dma_start(
                    out=mtiles[g][:],
                    in_=mask_in[int(goffs[g]):int(goffs[g + 1])]
                    .rearrange("(p f) -> p f", p=128))

            emit_group(0)
            for s, (k0, nk) in enumerate(windows):
                g, off = s2group[s]
                Ft = nk * W
                band_in = corr_flat[s * 128:(s + 1) * 128, k0:k0 + nk, :] \
                    .rearrange("r k l -> r (k l)")
                corr_tile = pool.tile([128, Ft], f32, tag="corr")
                nc.sync.dma_start(out=corr_tile[:], in_=band_in)
                if s + 1 < NTILE:
                    emit_group(s2group[s + 1][0])
                stage = pool.tile([128, Ft], f32, tag="stage")
                nc.vector.tensor_tensor(
                    out=stage[:], in0=corr_tile[:],
                    in1=mtiles[g][:, off:off + Ft],
                    op=mybir.AluOpType.mult)
                if s % 2 == 0:
                    scratch = pool.tile([128, Ft], f32, tag="scratch")
                    nc.scalar.activation(
                        out=scratch[:], in_=stage[:],
                        func=mybir.ActivationFunctionType.Copy,
                        accum_out=colsums[:, s:s + 1])
                else:
                    nc.vector.tensor_reduce(
                        out=colsums[:, s:s + 1], in_=stage[:],
                        axis=mybir.AxisListType.X, op=mybir.AluOpType.add)
                band_out = out_flat[s * 128:(s + 1) * 128, k0:k0 + nk, :] \
                    .rearrange("r k l -> r (k l)")
                nc.scalar.dma_start(out=band_out, in_=stage[:])

            rowacc = fpool.tile([128, 1], f32, tag="rowacc")
            nc.vector.tensor_reduce(out=rowacc[:], in_=colsums[:],
                                    axis=mybir.AxisListType.X,
                                    op=mybir.AluOpType.add)
            ones = cpool.tile([128, 1], f32, tag="ones")
            nc.vector.memset(ones[:], 1.0)
            ps = psump.tile([1, 1], f32, tag="ps")
            nc.tensor.matmul(ps[:], ones[:], rowacc[:], start=True, stop=True)
            fin = fpool.tile([1, 1], f32, tag="fin")
            nc.vector.tensor_copy(out=fin[:], in_=ps[:])
            nc.sync.dma_start(out=sum_t[:], in_=fin[:])

    nc.finalize()
    return nc


def kernel(geo_parameters, corr_scores):
    from concourse.bass_utils import run_bass_kernel_spmd

    geo_parameters = np.asarray(geo_parameters)
    corr_scores = np.ascontiguousarray(np.asarray(corr_scores, dtype=np.float32))

    gx, gy = _tps_grids_cpu(geo_parameters)
    windows, mask_flat, group_F = _build_plan(gx, gy)
    nc = _build_program(windows, group_F)

    in_maps = [{"corr": corr_scores[b], "mask": mask_flat[b]} for b in range(B)]
    res = run_bass_kernel_spmd(nc, in_maps, list(range(B)))
    global LAST_RESULTS
    LAST_RESULTS = res

    inlier = np.stack([res.results[b]["out"] for b in range(B)], axis=0)
    sums = np.array([res.results[b]["sums"][0, 0] for b in range(B)],
                    dtype=np.float32)
    return inlier, sums
